# revision 15
# baseline (speedup 1.0000x reference)
"""DiscriminativeLoss Trainium2 kernel (Bass/Tile), data-parallel over batch.

Per core: one batch element [N=131072, D=32] f32 + labels [N] i32.
Returns per-core partial losses (pull_b, push_b); host averages over the
8 cores and assembles [total, pull, push].

v2: all matmul sweeps are one LDW+MM pair per 128-point chunk:
  A: lhsT=oh_bf[128,32], rhs=[hi|lo|ones] F=65 -> segment sums (hi/lo
     bf16 split, ~2^-17 accurate) + exact counts.
  B: lhsT=ohT_full[32,128] (FWL bf16), rhs=means F=32 -> per-point mean.
  C: lhsT=oh_bf, rhs=hinge F=1 -> per-label hinge sums.
"""

import os
import sys

sys.path.insert(0, "/opt/trn_rl_repo")

import numpy as np
from contextlib import ExitStack

import concourse.bass as bass
import concourse.bacc as bacc
import concourse.mybir as mybir
import concourse.tile as tile

F32 = mybir.dt.float32
BF16 = mybir.dt.bfloat16
I32 = mybir.dt.int32
AX = mybir.AxisListType
OP = mybir.AluOpType
AF = mybir.ActivationFunctionType

B, N_FULL, D = 8, 131072, 32
EMIT_PHASE = "full"   # "a" | "ab" | "abc" | "full"  (bisect aid)
NL = 32          # instance labels 1..32 (label 0 ignored)
DELTA_V = 0.1
DELTA_D = 0.5
HL = 2 * D + 2   # 66: hi(32) | lo(32) | ones(1) | pad, 4B-aligned stride


def emit(tc, emb_d, lab_d, res_d, groups):
    nc = tc.nc
    ctx = tc.ctx
    npc = groups * 32           # points per partition

    emb_v = emb_d[:].rearrange("(p c) d -> p (c d)", p=128)
    lab_v = lab_d[:].rearrange("(p c) -> p c", p=128)

    # ---------------- pools ----------------
    p_in = ctx.enter_context(tc.tile_pool(name="p_in", bufs=3))
    p_ohb = ctx.enter_context(tc.tile_pool(name="p_ohb", bufs=3))
    p_oht = ctx.enter_context(tc.tile_pool(name="p_oht", bufs=2))
    p_ohf = ctx.enter_context(tc.tile_pool(name="p_ohf", bufs=2))
    p_pers = ctx.enter_context(tc.tile_pool(name="p_pers", bufs=1))
    p_small = ctx.enter_context(tc.tile_pool(name="p_small", bufs=1))
    p_dve = ctx.enter_context(tc.tile_pool(name="p_dve", bufs=3))
    ps_a = ctx.enter_context(tc.tile_pool(name="ps_a", bufs=1, space="PSUM"))
    ps_hc = ctx.enter_context(tc.tile_pool(name="ps_hc", bufs=1, space="PSUM"))
    ps_mp = ctx.enter_context(tc.tile_pool(name="ps_mp", bufs=2, space="PSUM"))
    ps_misc = ctx.enter_context(tc.tile_pool(name="ps_misc", bufs=3, space="PSUM"))

    # ---------------- constants / persistent ----------------
    lab_i = p_in.tile([128, npc], I32, tag="lab_i")
    nc.sync.dma_start(lab_i[:], lab_v)
    lab_b = p_pers.tile([128, npc], BF16, tag="lab_b")
    nc.vector.tensor_copy(lab_b[:], lab_i[:])

    iota_i = p_small.tile([128, NL], I32, tag="iota_i")
    nc.gpsimd.iota(iota_i[:], pattern=[[1, NL]], base=1, channel_multiplier=0)
    iota_b = p_small.tile([128, NL], BF16, tag="iota_b")
    nc.vector.tensor_copy(iota_b[:], iota_i[:])

    # 32x32 identity (f32)
    ones32 = p_small.tile([32, 32], F32, tag="ones32")
    nc.vector.memset(ones32[:], 1.0)
    id32 = p_small.tile([32, 32], F32, tag="id32")
    nc.gpsimd.affine_select(
        id32[:], ones32[:], pattern=[[1, 32]], base=0,
        channel_multiplier=-1, compare_op=OP.is_equal, fill=0.0,
    )
    ones_k1 = p_small.tile([1, 32], F32, tag="ones_k1")
    nc.vector.memset(ones_k1[:], 1.0)
    ones32c = p_small.tile([32, 1], F32, tag="ones32c")
    nc.vector.memset(ones32c[:], 1.0)
    eps_b = p_small.tile([128, 1], F32, tag="eps_b")
    nc.vector.memset(eps_b[:], 1e-24)
    ndv_b = p_small.tile([128, 1], F32, tag="ndv_b")
    nc.vector.memset(ndv_b[:], -DELTA_V)

    # persistent big buffers
    emb_hl = p_pers.tile([128, npc * HL], BF16, tag="emb_hl")
    h_all = p_pers.tile([128, npc], BF16, tag="h_all")

    psum_a = ps_a.tile([32, HL], F32, tag="psum_a")
    psum_hc = ps_hc.tile([32, 1], F32, tag="psum_hc")

    # strided views of emb_hl: [p, point, slot]
    hl3 = emb_hl[:].rearrange("p (c k) -> p c k", k=HL)

    # ones (col 64) + pad (col 65) of each 66-block, set once
    nc.vector.memset(hl3[:, :, 2 * D:2 * D + 2], 1.0)

    # ================= PHASE A: segment sums + counts =================
    for g in range(groups):
        ta = p_in.tile([128, 1024], F32, tag="ta")
        nc.sync.dma_start(ta[:], emb_v[:, g * 1024:(g + 1) * 1024])
        ta3 = ta[:].rearrange("p (c d) -> p c d", d=D)

        oh = p_ohb.tile([128, 1024], BF16, tag="ohb")
        in0 = lab_b[:, g * 32:(g + 1) * 32].unsqueeze(2).broadcast_to([128, 32, NL])
        in1 = iota_b[:].unsqueeze(1).broadcast_to([128, 32, NL])
        oh3 = oh[:].rearrange("p (j l) -> p j l", l=NL)
        nc.vector.tensor_tensor(out=oh3, in0=in0, in1=in1, op=OP.is_equal)

        # hi = bf16(e) (ACT), lo = bf16(e - hi) (DVE)
        hi3 = hl3[:, g * 32:(g + 1) * 32, 0:D]
        lo3 = hl3[:, g * 32:(g + 1) * 32, D:2 * D]
        nc.scalar.copy(hi3, ta3)
        nc.gpsimd.tensor_tensor(out=lo3, in0=ta3, in1=hi3, op=OP.subtract)

        for j in range(32):
            cj = g * 32 + j
            nc.tensor.matmul(
                psum_a[:], oh[:, j * NL:(j + 1) * NL],
                emb_hl[:, cj * HL:(cj + 1) * HL],
                start=(cj == 0), stop=(cj == groups * 32 - 1),
            )

    # ================= means & push tail (tiny, f32) =================
    cnt = psum_a[:, 2 * D:2 * D + 1]
    cnt_cl = p_small.tile([32, 1], F32, tag="cnt_cl")
    nc.vector.tensor_scalar(out=cnt_cl[:], in0=cnt, scalar1=1.0,
                            scalar2=None, op0=OP.max)
    recip = p_small.tile([32, 1], F32, tag="recip")
    nc.vector.reciprocal(recip[:], cnt_cl[:])
    suml_sb = p_small.tile([32, 32], F32, tag="suml_sb")
    nc.vector.tensor_copy(suml_sb[:], psum_a[:, D:2 * D])
    sums_f = p_small.tile([32, 32], F32, tag="sums_f")
    nc.vector.tensor_tensor(out=sums_f[:], in0=psum_a[:, 0:D],
                            in1=suml_sb[:], op=OP.add)
    means_f = p_small.tile([32, 32], F32, tag="means_f")
    nc.vector.tensor_scalar(out=means_f[:], in0=sums_f[:],
                            scalar1=recip[:], scalar2=None, op0=OP.mult)
    means_b = p_small.tile([32, 32], BF16, tag="means_b")
    nc.vector.tensor_copy(means_b[:], means_f[:])

    if EMIT_PHASE == "a":
        res_sb = p_small.tile([1, 8], F32, tag="res_sb")
        nc.vector.memset(res_sb[:], 0.0)
        nc.vector.tensor_copy(res_sb[:, 0:1], cnt_cl[0:1, :])
        nc.sync.dma_start(res_d[:], res_sb[:])
        return

    # --- push loss on the 32x32 mean matrix ---
    mnsq = p_small.tile([32, 32], F32, tag="mnsq")
    nc.vector.tensor_tensor(out=mnsq[:], in0=means_f[:], in1=means_f[:], op=OP.mult)
    nrm2 = p_small.tile([32, 1], F32, tag="nrm2")
    nc.vector.reduce_sum(out=nrm2[:], in_=mnsq[:], axis=AX.X)
    nrm = p_small.tile([32, 1], F32, tag="nrm")
    nc.scalar.activation(nrm[:], nrm2[:], AF.Sqrt)
    nrm_cl = p_small.tile([32, 1], F32, tag="nrm_cl")
    nc.vector.tensor_scalar(out=nrm_cl[:], in0=nrm[:], scalar1=1e-12,
                            scalar2=None, op0=OP.max)
    rnrm = p_small.tile([32, 1], F32, tag="rnrm")
    nc.vector.reciprocal(rnrm[:], nrm_cl[:])
    mn = p_small.tile([32, 32], F32, tag="mn")
    nc.vector.tensor_scalar(out=mn[:], in0=means_f[:], scalar1=rnrm[:],
                            scalar2=None, op0=OP.mult)

    ps_mnt = ps_misc.tile([32, 32], F32, tag="misc")
    nc.tensor.transpose(ps_mnt[:], mn[:], id32[:])
    mnt = p_small.tile([32, 32], F32, tag="mnt")
    nc.vector.tensor_copy(mnt[:], ps_mnt[:])

    ps_g = ps_misc.tile([32, 32], F32, tag="misc")
    nc.tensor.matmul(ps_g[:], mnt[:], mnt[:], start=True, stop=True)

    mnsq2 = p_small.tile([32, 32], F32, tag="mnsq2")
    nc.vector.tensor_tensor(out=mnsq2[:], in0=mn[:], in1=mn[:], op=OP.mult)
    nsq = p_small.tile([32, 1], F32, tag="nsq")
    nc.vector.reduce_sum(out=nsq[:], in_=mnsq2[:], axis=AX.X)

    present = p_small.tile([32, 1], F32, tag="present")
    nc.vector.tensor_scalar(out=present[:], in0=cnt, scalar1=0.0,
                            scalar2=None, op0=OP.is_gt)

    sq_a = p_small.tile([32, 32], F32, tag="sq_a")
    nc.vector.tensor_scalar(out=sq_a[:], in0=ps_g[:], scalar1=-2.0,
                            scalar2=nsq[:], op0=OP.mult, op1=OP.add)

    ps_row0 = ps_misc.tile([1, 32], F32, tag="misc")
    nc.tensor.matmul(ps_row0[:], nsq[:], id32[:], start=True, stop=True)
    nsqt_sb = p_small.tile([1, 32], F32, tag="nsqt_sb")
    nc.vector.tensor_copy(nsqt_sb[:], ps_row0[:])
    ps_row1 = ps_misc.tile([1, 32], F32, tag="misc")
    nc.tensor.matmul(ps_row1[:], present[:], id32[:], start=True, stop=True)
    prest_sb = p_small.tile([1, 32], F32, tag="prest_sb")
    nc.vector.tensor_copy(prest_sb[:], ps_row1[:])

    ps_bc = ps_misc.tile([32, 64], F32, tag="misc")
    nc.tensor.matmul(ps_bc[:, 0:32], ones_k1[:], nsqt_sb[:],
                     start=True, stop=True)
    nc.tensor.matmul(ps_bc[:, 32:64], ones_k1[:], prest_sb[:],
                     start=True, stop=True)
    nsq_j = p_small.tile([32, 32], F32, tag="nsq_j")
    nc.vector.tensor_copy(nsq_j[:], ps_bc[:, 0:32])
    pres_j = p_small.tile([32, 32], F32, tag="pres_j")
    nc.vector.tensor_copy(pres_j[:], ps_bc[:, 32:64])

    sq0 = p_small.tile([32, 32], F32, tag="sq0")
    nc.vector.tensor_tensor(out=sq0[:], in0=sq_a[:], in1=nsq_j[:], op=OP.add)
    sq = p_small.tile([32, 32], F32, tag="sq")
    nc.vector.tensor_scalar(out=sq[:], in0=sq0[:], scalar1=0.0,
                            scalar2=None, op0=OP.max)
    dmat = p_small.tile([32, 32], F32, tag="dmat")
    nc.scalar.activation(dmat[:], sq[:], AF.Sqrt, bias=eps_b[0:32, :])
    hp0 = p_small.tile([32, 32], F32, tag="hp0")
    nc.scalar.activation(hp0[:], dmat[:], AF.Relu, bias=ones32c[:], scale=-1.0)
    hp1 = p_small.tile([32, 32], F32, tag="hp1")
    nc.vector.tensor_scalar(out=hp1[:], in0=hp0[:], scalar1=present[:],
                            scalar2=None, op0=OP.mult)
    hp2 = p_small.tile([32, 32], F32, tag="hp2")
    nc.vector.tensor_tensor(out=hp2[:], in0=hp1[:], in1=pres_j[:], op=OP.mult)
    hp3 = p_small.tile([32, 32], F32, tag="hp3")
    nc.gpsimd.affine_select(hp3[:], hp2[:], pattern=[[1, 32]], base=0,
                            channel_multiplier=-1, compare_op=OP.is_gt, fill=0.0)
    pm1 = p_small.tile([32, 32], F32, tag="pm1")
    nc.vector.tensor_scalar(out=pm1[:], in0=pres_j[:], scalar1=present[:],
                            scalar2=None, op0=OP.mult)
    pm = p_small.tile([32, 32], F32, tag="pm")
    nc.gpsimd.affine_select(pm[:], pm1[:], pattern=[[1, 32]], base=0,
                            channel_multiplier=-1, compare_op=OP.is_gt, fill=0.0)
    hp_rs = p_small.tile([32, 1], F32, tag="hp_rs")
    nc.vector.reduce_sum(out=hp_rs[:], in_=hp3[:], axis=AX.X)
    pm_rs = p_small.tile([32, 1], F32, tag="pm_rs")
    nc.vector.reduce_sum(out=pm_rs[:], in_=pm[:], axis=AX.X)

    # ================= PHASE B (+ pipelined C): pull loss =================
    prev_oh = None
    prev_g = None
    first_c = [True]

    def emit_phase_c(ohb_t, g):
        for j in range(32):
            cj = g * 32 + j
            nc.tensor.matmul(
                psum_hc[:], ohb_t[:, j * NL:(j + 1) * NL],
                h_all[:, cj:cj + 1],
                start=first_c[0], stop=(cj == groups * 32 - 1),
            )
            first_c[0] = False

    for g in range(groups):
        ohb = p_ohb.tile([128, 1024], BF16, tag="ohb")
        in0 = lab_b[:, g * 32:(g + 1) * 32].unsqueeze(2).broadcast_to([128, 32, NL])
        in1 = iota_b[:].unsqueeze(1).broadcast_to([128, 32, NL])
        ohb3 = ohb[:].rearrange("p (j l) -> p j l", l=NL)
        nc.vector.tensor_tensor(out=ohb3, in0=in0, in1=in1, op=OP.is_equal)

        oht = p_oht.tile([128, 1024], BF16, tag="oht")
        nc.vector.transpose(oht[:], ohb[:])

        # rearrange 32x32 blocks: ohT_full[0:32, 128j+32b+s] <- ohT block
        ohf = p_ohf.tile([32, 4096], BF16, tag="ohf")
        ohf3 = ohf[:].rearrange("p (j b s) -> p j b s", b=4, s=32)
        oht3 = oht[:].rearrange("p (j s) -> p j s", s=32)
        for b4 in range(4):
            nc.sync.dma_start(ohf3[:, :, b4, :],
                              oht3[32 * b4:32 * b4 + 32, :, :])

        for half in range(2):
            mp = ps_mp.tile([128, 512], F32, tag="mp")
            for jj in range(16):
                j = half * 16 + jj
                nc.tensor.matmul(
                    mp[:, jj * 32:(jj + 1) * 32],
                    ohf[:, j * 128:(j + 1) * 128],
                    means_b[:],
                    start=True, stop=True,
                )
            base3 = hl3[:, g * 32 + half * 16:g * 32 + (half + 1) * 16, 0:D]
            diff = p_dve.tile([128, 512], BF16, tag="diff")
            diff3 = diff[:].rearrange("p (j d) -> p j d", d=D)
            nc.vector.tensor_tensor(out=diff3, in0=base3, in1=mp[:].rearrange(
                "p (j d) -> p j d", d=D), op=OP.subtract)
            sqd = p_dve.tile([128, 512], BF16, tag="sqd")
            nc.scalar.activation(sqd[:], diff[:], AF.Square)
            d2 = p_dve.tile([128, 16], F32, tag="d2")
            nc.vector.reduce_sum(
                out=d2[:], in_=sqd[:].rearrange("p (j d) -> p j d", d=D),
                axis=AX.X,
            )
            dist = p_dve.tile([128, 16], F32, tag="dist")
            nc.scalar.activation(dist[:], d2[:], AF.Sqrt, bias=eps_b[:])
            hcol = g * 32 + half * 16
            nc.scalar.activation(h_all[:, hcol:hcol + 16], dist[:],
                                 AF.Relu, bias=ndv_b[:])

        if EMIT_PHASE != "ab" and prev_oh is not None:
            emit_phase_c(prev_oh, prev_g)
        prev_oh, prev_g = ohb, g
    if EMIT_PHASE == "ab":
        res_sb = p_small.tile([1, 8], F32, tag="res_sb")
        nc.vector.memset(res_sb[:], 0.0)
        nc.vector.tensor_copy(res_sb[:, 0:1], h_all[0:1, 0:1])
        nc.sync.dma_start(res_d[:], res_sb[:])
        return
    emit_phase_c(prev_oh, prev_g)

    if EMIT_PHASE == "abc":
        res_sb = p_small.tile([1, 8], F32, tag="res_sb")
        nc.vector.memset(res_sb[:], 0.0)
        nc.vector.tensor_copy(res_sb[:, 0:1], psum_hc[0:1, :])
        nc.sync.dma_start(res_d[:], res_sb[:])
        return

    # ================= finals =================
    seg_mean = p_small.tile([32, 1], F32, tag="seg_mean")
    nc.vector.tensor_scalar(out=seg_mean[:], in0=psum_hc[:], scalar1=recip[:],
                            scalar2=None, op0=OP.mult)

    cat4 = p_small.tile([32, 4], F32, tag="cat4")
    nc.vector.tensor_copy(cat4[:, 0:1], seg_mean[:])
    nc.vector.tensor_copy(cat4[:, 1:2], present[:])
    nc.vector.tensor_copy(cat4[:, 2:3], hp_rs[:])
    nc.vector.tensor_copy(cat4[:, 3:4], pm_rs[:])
    ps_fin = ps_misc.tile([1, 4], F32, tag="misc")
    nc.tensor.matmul(ps_fin[:], ones32c[:], cat4[:], start=True, stop=True)
    sc = p_small.tile([1, 4], F32, tag="sc")
    nc.vector.tensor_copy(sc[:], ps_fin[:])

    res_sb = p_small.tile([1, 8], F32, tag="res_sb")
    nc.vector.memset(res_sb[:], 0.0)
    t1 = p_small.tile([1, 1], F32, tag="t1")
    nc.vector.tensor_scalar(out=t1[:], in0=sc[:, 1:2], scalar1=1e-6,
                            scalar2=None, op0=OP.add)
    r1 = p_small.tile([1, 1], F32, tag="r1")
    nc.vector.reciprocal(r1[:], t1[:])
    nc.vector.tensor_tensor(out=res_sb[:, 0:1], in0=sc[:, 0:1], in1=r1[:],
                            op=OP.mult)
    t2 = p_small.tile([1, 1], F32, tag="t2")
    nc.vector.tensor_scalar(out=t2[:], in0=sc[:, 3:4], scalar1=1e-6,
                            scalar2=None, op0=OP.add)
    r2 = p_small.tile([1, 1], F32, tag="r2")
    nc.vector.reciprocal(r2[:], t2[:])
    pb0 = p_small.tile([1, 1], F32, tag="pb0")
    nc.vector.tensor_tensor(out=pb0[:], in0=sc[:, 2:3], in1=r2[:], op=OP.mult)
    gate = p_small.tile([1, 1], F32, tag="gate")
    nc.vector.tensor_scalar(out=gate[:], in0=sc[:, 1:2], scalar1=1.0,
                            scalar2=None, op0=OP.is_gt)
    nc.vector.tensor_tensor(out=res_sb[:, 1:2], in0=pb0[:], in1=gate[:],
                            op=OP.mult)

    nc.sync.dma_start(res_d[:], res_sb[:])


def build_program(groups):
    n = groups * 4096
    nc = bacc.Bacc("TRN2", target_bir_lowering=False, debug=False)
    emb_d = nc.dram_tensor("emb", [n, D], F32, kind="ExternalInput")
    lab_d = nc.dram_tensor("lab", [n], I32, kind="ExternalInput")
    res_d = nc.dram_tensor("res", [1, 8], F32, kind="ExternalOutput")
    with tile.TileContext(nc) as tc:
        with ExitStack() as ctx:
            tc.ctx = ctx
            emit(tc, emb_d, lab_d, res_d, groups)
    nc.compile()
    return nc


_NC_CACHE = {}


def _get_nc(groups):
    if groups not in _NC_CACHE:
        _NC_CACHE[groups] = build_program(groups)
    return _NC_CACHE[groups]


def kernel(embeddings, labels):
    embeddings = np.asarray(embeddings, dtype=np.float32)
    labels = np.asarray(labels, dtype=np.int32)
    bsz = embeddings.shape[0]
    groups = embeddings.shape[1] // 4096
    nc = _get_nc(groups)

    from concourse.bass_utils import run_bass_kernel_spmd

    in_maps = [
        {"emb": np.ascontiguousarray(embeddings[b]),
         "lab": np.ascontiguousarray(labels[b])}
        for b in range(bsz)
    ]
    out = run_bass_kernel_spmd(nc, in_maps, list(range(bsz)))
    res = np.stack([out.results[b]["res"][0] for b in range(bsz)])
    pull = res[:, 0].sum() / bsz
    push = res[:, 1].sum() / bsz
    return np.stack([pull + push, pull, push]).astype(np.float32)


# revision 17
# speedup vs baseline: 1.0294x; 1.0294x over previous
"""DiscriminativeLoss Trainium2 kernel (Bass/Tile), data-parallel over batch.

Per core: one batch element [N=131072, D=32] f32 + labels [N] i32.
Returns per-core partial losses (pull_b, push_b); host averages over the
8 cores and assembles [total, pull, push].

v2: all matmul sweeps are one LDW+MM pair per 128-point chunk:
  A: lhsT=oh_bf[128,32], rhs=[hi|lo|ones] F=65 -> segment sums (hi/lo
     bf16 split, ~2^-17 accurate) + exact counts.
  B: lhsT=ohT_full[32,128] (FWL bf16), rhs=means F=32 -> per-point mean.
  C: lhsT=oh_bf, rhs=hinge F=1 -> per-label hinge sums.
"""

import os
import sys

sys.path.insert(0, "/opt/trn_rl_repo")

import numpy as np
from contextlib import ExitStack

import concourse.bass as bass
import concourse.bacc as bacc
import concourse.mybir as mybir
import concourse.tile as tile

F32 = mybir.dt.float32
BF16 = mybir.dt.bfloat16
I32 = mybir.dt.int32
AX = mybir.AxisListType
OP = mybir.AluOpType
AF = mybir.ActivationFunctionType

B, N_FULL, D = 8, 131072, 32
EMIT_PHASE = "full"   # "a" | "ab" | "abc" | "full"  (bisect aid)
NL = 32          # instance labels 1..32 (label 0 ignored)
DELTA_V = 0.1
DELTA_D = 0.5
HL = 2 * D + 2   # 66: hi(32) | lo(32) | ones(1) | pad, 4B-aligned stride


def emit(tc, emb_d, lab_d, res_d, groups):
    nc = tc.nc
    ctx = tc.ctx
    npc = groups * 32           # points per partition

    emb_v = emb_d[:].rearrange("(p c) d -> p (c d)", p=128)
    lab_v = lab_d[:].rearrange("(p c) -> p c", p=128)

    # ---------------- pools ----------------
    p_in = ctx.enter_context(tc.tile_pool(name="p_in", bufs=3))
    p_ohb = ctx.enter_context(tc.tile_pool(name="p_ohb", bufs=3))
    p_oht = ctx.enter_context(tc.tile_pool(name="p_oht", bufs=3))
    p_ohf = ctx.enter_context(tc.tile_pool(name="p_ohf", bufs=3))
    p_pers = ctx.enter_context(tc.tile_pool(name="p_pers", bufs=1))
    p_small = ctx.enter_context(tc.tile_pool(name="p_small", bufs=1))
    p_dve = ctx.enter_context(tc.tile_pool(name="p_dve", bufs=3))
    ps_a = ctx.enter_context(tc.tile_pool(name="ps_a", bufs=1, space="PSUM"))
    ps_hc = ctx.enter_context(tc.tile_pool(name="ps_hc", bufs=1, space="PSUM"))
    ps_mp = ctx.enter_context(tc.tile_pool(name="ps_mp", bufs=2, space="PSUM"))
    ps_misc = ctx.enter_context(tc.tile_pool(name="ps_misc", bufs=3, space="PSUM"))

    # ---------------- constants / persistent ----------------
    lab_i = p_in.tile([128, npc], I32, tag="lab_i")
    nc.sync.dma_start(lab_i[:], lab_v)
    lab_b = p_pers.tile([128, npc], BF16, tag="lab_b")
    nc.vector.tensor_copy(lab_b[:], lab_i[:])

    iota_i = p_small.tile([128, NL], I32, tag="iota_i")
    nc.gpsimd.iota(iota_i[:], pattern=[[1, NL]], base=1, channel_multiplier=0)
    iota_b = p_small.tile([128, NL], BF16, tag="iota_b")
    nc.vector.tensor_copy(iota_b[:], iota_i[:])

    # 32x32 identity (f32)
    ones32 = p_small.tile([32, 32], F32, tag="ones32")
    nc.vector.memset(ones32[:], 1.0)
    id32 = p_small.tile([32, 32], F32, tag="id32")
    nc.gpsimd.affine_select(
        id32[:], ones32[:], pattern=[[1, 32]], base=0,
        channel_multiplier=-1, compare_op=OP.is_equal, fill=0.0,
    )
    ones_k1 = p_small.tile([1, 32], F32, tag="ones_k1")
    nc.vector.memset(ones_k1[:], 1.0)
    ones32c = p_small.tile([32, 1], F32, tag="ones32c")
    nc.vector.memset(ones32c[:], 1.0)
    eps_b = p_small.tile([128, 1], F32, tag="eps_b")
    nc.vector.memset(eps_b[:], 1e-24)
    ndv_b = p_small.tile([128, 1], F32, tag="ndv_b")
    nc.vector.memset(ndv_b[:], -DELTA_V)

    # persistent big buffers
    emb_hl = p_pers.tile([128, npc * HL], BF16, tag="emb_hl")
    h_all = p_pers.tile([128, npc], BF16, tag="h_all")

    psum_a = ps_a.tile([32, HL], F32, tag="psum_a")
    psum_hc = ps_hc.tile([32, 1], F32, tag="psum_hc")

    # strided views of emb_hl: [p, point, slot]
    hl3 = emb_hl[:].rearrange("p (c k) -> p c k", k=HL)

    # ones (col 64) + pad (col 65) of each 66-block, set once
    nc.vector.memset(hl3[:, :, 2 * D:2 * D + 2], 1.0)

    # ================= PHASE A: segment sums + counts =================
    for g in range(groups):
        ta = p_in.tile([128, 1024], F32, tag="ta")
        nc.sync.dma_start(ta[:], emb_v[:, g * 1024:(g + 1) * 1024])
        ta3 = ta[:].rearrange("p (c d) -> p c d", d=D)

        oh = p_ohb.tile([128, 1024], BF16, tag="ohb")
        in0 = lab_b[:, g * 32:(g + 1) * 32].unsqueeze(2).broadcast_to([128, 32, NL])
        in1 = iota_b[:].unsqueeze(1).broadcast_to([128, 32, NL])
        oh3 = oh[:].rearrange("p (j l) -> p j l", l=NL)
        nc.vector.tensor_tensor(out=oh3, in0=in0, in1=in1, op=OP.is_equal)

        # hi = bf16(e) (ACT), lo = bf16(e - hi) (DVE)
        hi3 = hl3[:, g * 32:(g + 1) * 32, 0:D]
        lo3 = hl3[:, g * 32:(g + 1) * 32, D:2 * D]
        nc.scalar.copy(hi3, ta3)
        nc.vector.tensor_tensor(out=lo3, in0=ta3, in1=hi3, op=OP.subtract)

        for j in range(32):
            cj = g * 32 + j
            nc.tensor.matmul(
                psum_a[:], oh[:, j * NL:(j + 1) * NL],
                emb_hl[:, cj * HL:(cj + 1) * HL],
                start=(cj == 0), stop=(cj == groups * 32 - 1),
            )

    # ================= means & push tail (tiny, f32) =================
    cnt = psum_a[:, 2 * D:2 * D + 1]
    cnt_cl = p_small.tile([32, 1], F32, tag="cnt_cl")
    nc.vector.tensor_scalar(out=cnt_cl[:], in0=cnt, scalar1=1.0,
                            scalar2=None, op0=OP.max)
    recip = p_small.tile([32, 1], F32, tag="recip")
    nc.vector.reciprocal(recip[:], cnt_cl[:])
    suml_sb = p_small.tile([32, 32], F32, tag="suml_sb")
    nc.vector.tensor_copy(suml_sb[:], psum_a[:, D:2 * D])
    sums_f = p_small.tile([32, 32], F32, tag="sums_f")
    nc.vector.tensor_tensor(out=sums_f[:], in0=psum_a[:, 0:D],
                            in1=suml_sb[:], op=OP.add)
    means_f = p_small.tile([32, 32], F32, tag="means_f")
    nc.vector.tensor_scalar(out=means_f[:], in0=sums_f[:],
                            scalar1=recip[:], scalar2=None, op0=OP.mult)
    means_b = p_small.tile([32, 32], BF16, tag="means_b")
    nc.vector.tensor_copy(means_b[:], means_f[:])

    if EMIT_PHASE == "a":
        res_sb = p_small.tile([1, 8], F32, tag="res_sb")
        nc.vector.memset(res_sb[:], 0.0)
        nc.vector.tensor_copy(res_sb[:, 0:1], cnt_cl[0:1, :])
        nc.sync.dma_start(res_d[:], res_sb[:])
        return

    # --- push loss on the 32x32 mean matrix ---
    mnsq = p_small.tile([32, 32], F32, tag="mnsq")
    nc.vector.tensor_tensor(out=mnsq[:], in0=means_f[:], in1=means_f[:], op=OP.mult)
    nrm2 = p_small.tile([32, 1], F32, tag="nrm2")
    nc.vector.reduce_sum(out=nrm2[:], in_=mnsq[:], axis=AX.X)
    nrm = p_small.tile([32, 1], F32, tag="nrm")
    nc.scalar.activation(nrm[:], nrm2[:], AF.Sqrt)
    nrm_cl = p_small.tile([32, 1], F32, tag="nrm_cl")
    nc.vector.tensor_scalar(out=nrm_cl[:], in0=nrm[:], scalar1=1e-12,
                            scalar2=None, op0=OP.max)
    rnrm = p_small.tile([32, 1], F32, tag="rnrm")
    nc.vector.reciprocal(rnrm[:], nrm_cl[:])
    mn = p_small.tile([32, 32], F32, tag="mn")
    nc.vector.tensor_scalar(out=mn[:], in0=means_f[:], scalar1=rnrm[:],
                            scalar2=None, op0=OP.mult)

    ps_mnt = ps_misc.tile([32, 32], F32, tag="misc")
    nc.tensor.transpose(ps_mnt[:], mn[:], id32[:])
    mnt = p_small.tile([32, 32], F32, tag="mnt")
    nc.vector.tensor_copy(mnt[:], ps_mnt[:])

    ps_g = ps_misc.tile([32, 32], F32, tag="misc")
    nc.tensor.matmul(ps_g[:], mnt[:], mnt[:], start=True, stop=True)

    mnsq2 = p_small.tile([32, 32], F32, tag="mnsq2")
    nc.vector.tensor_tensor(out=mnsq2[:], in0=mn[:], in1=mn[:], op=OP.mult)
    nsq = p_small.tile([32, 1], F32, tag="nsq")
    nc.vector.reduce_sum(out=nsq[:], in_=mnsq2[:], axis=AX.X)

    present = p_small.tile([32, 1], F32, tag="present")
    nc.vector.tensor_scalar(out=present[:], in0=cnt, scalar1=0.0,
                            scalar2=None, op0=OP.is_gt)

    sq_a = p_small.tile([32, 32], F32, tag="sq_a")
    nc.vector.tensor_scalar(out=sq_a[:], in0=ps_g[:], scalar1=-2.0,
                            scalar2=nsq[:], op0=OP.mult, op1=OP.add)

    ps_row0 = ps_misc.tile([1, 32], F32, tag="misc")
    nc.tensor.matmul(ps_row0[:], nsq[:], id32[:], start=True, stop=True)
    nsqt_sb = p_small.tile([1, 32], F32, tag="nsqt_sb")
    nc.vector.tensor_copy(nsqt_sb[:], ps_row0[:])
    ps_row1 = ps_misc.tile([1, 32], F32, tag="misc")
    nc.tensor.matmul(ps_row1[:], present[:], id32[:], start=True, stop=True)
    prest_sb = p_small.tile([1, 32], F32, tag="prest_sb")
    nc.vector.tensor_copy(prest_sb[:], ps_row1[:])

    ps_bc = ps_misc.tile([32, 64], F32, tag="misc")
    nc.tensor.matmul(ps_bc[:, 0:32], ones_k1[:], nsqt_sb[:],
                     start=True, stop=True)
    nc.tensor.matmul(ps_bc[:, 32:64], ones_k1[:], prest_sb[:],
                     start=True, stop=True)
    nsq_j = p_small.tile([32, 32], F32, tag="nsq_j")
    nc.vector.tensor_copy(nsq_j[:], ps_bc[:, 0:32])
    pres_j = p_small.tile([32, 32], F32, tag="pres_j")
    nc.vector.tensor_copy(pres_j[:], ps_bc[:, 32:64])

    sq0 = p_small.tile([32, 32], F32, tag="sq0")
    nc.vector.tensor_tensor(out=sq0[:], in0=sq_a[:], in1=nsq_j[:], op=OP.add)
    sq = p_small.tile([32, 32], F32, tag="sq")
    nc.vector.tensor_scalar(out=sq[:], in0=sq0[:], scalar1=0.0,
                            scalar2=None, op0=OP.max)
    dmat = p_small.tile([32, 32], F32, tag="dmat")
    nc.scalar.activation(dmat[:], sq[:], AF.Sqrt, bias=eps_b[0:32, :])
    hp0 = p_small.tile([32, 32], F32, tag="hp0")
    nc.scalar.activation(hp0[:], dmat[:], AF.Relu, bias=ones32c[:], scale=-1.0)
    hp1 = p_small.tile([32, 32], F32, tag="hp1")
    nc.vector.tensor_scalar(out=hp1[:], in0=hp0[:], scalar1=present[:],
                            scalar2=None, op0=OP.mult)
    hp2 = p_small.tile([32, 32], F32, tag="hp2")
    nc.vector.tensor_tensor(out=hp2[:], in0=hp1[:], in1=pres_j[:], op=OP.mult)
    hp3 = p_small.tile([32, 32], F32, tag="hp3")
    nc.gpsimd.affine_select(hp3[:], hp2[:], pattern=[[1, 32]], base=0,
                            channel_multiplier=-1, compare_op=OP.is_gt, fill=0.0)
    pm1 = p_small.tile([32, 32], F32, tag="pm1")
    nc.vector.tensor_scalar(out=pm1[:], in0=pres_j[:], scalar1=present[:],
                            scalar2=None, op0=OP.mult)
    pm = p_small.tile([32, 32], F32, tag="pm")
    nc.gpsimd.affine_select(pm[:], pm1[:], pattern=[[1, 32]], base=0,
                            channel_multiplier=-1, compare_op=OP.is_gt, fill=0.0)
    hp_rs = p_small.tile([32, 1], F32, tag="hp_rs")
    nc.vector.reduce_sum(out=hp_rs[:], in_=hp3[:], axis=AX.X)
    pm_rs = p_small.tile([32, 1], F32, tag="pm_rs")
    nc.vector.reduce_sum(out=pm_rs[:], in_=pm[:], axis=AX.X)

    # ================= PHASE B (+ pipelined C): pull loss =================
    prev_oh = None
    prev_g = None
    first_c = [True]

    def emit_phase_c(ohb_t, g):
        for j in range(32):
            cj = g * 32 + j
            nc.tensor.matmul(
                psum_hc[:], ohb_t[:, j * NL:(j + 1) * NL],
                h_all[:, cj:cj + 1],
                start=first_c[0], stop=(cj == groups * 32 - 1),
            )
            first_c[0] = False

    for g in range(groups):
        ohb = p_ohb.tile([128, 1024], BF16, tag="ohb")
        in0 = lab_b[:, g * 32:(g + 1) * 32].unsqueeze(2).broadcast_to([128, 32, NL])
        in1 = iota_b[:].unsqueeze(1).broadcast_to([128, 32, NL])
        ohb3 = ohb[:].rearrange("p (j l) -> p j l", l=NL)
        nc.vector.tensor_tensor(out=ohb3, in0=in0, in1=in1, op=OP.is_equal)

        oht = p_oht.tile([128, 1024], BF16, tag="oht")
        nc.vector.transpose(oht[:], ohb[:])

        # rearrange 32x32 blocks: ohT_full[0:32, 128j+32b+s] <- ohT block
        ohf = p_ohf.tile([32, 4096], BF16, tag="ohf")
        ohf3 = ohf[:].rearrange("p (j b s) -> p j b s", b=4, s=32)
        oht3 = oht[:].rearrange("p (j s) -> p j s", s=32)
        for b4 in range(4):
            nc.sync.dma_start(ohf3[:, :, b4, :],
                              oht3[32 * b4:32 * b4 + 32, :, :])

        for half in range(2):
            mp = ps_mp.tile([128, 512], F32, tag="mp")
            for jj in range(16):
                j = half * 16 + jj
                nc.tensor.matmul(
                    mp[:, jj * 32:(jj + 1) * 32],
                    ohf[:, j * 128:(j + 1) * 128],
                    means_b[:],
                    start=True, stop=True,
                )
            base3 = hl3[:, g * 32 + half * 16:g * 32 + (half + 1) * 16, 0:D]
            diff = p_dve.tile([128, 512], BF16, tag="diff")
            diff3 = diff[:].rearrange("p (j d) -> p j d", d=D)
            nc.vector.tensor_tensor(out=diff3, in0=base3, in1=mp[:].rearrange(
                "p (j d) -> p j d", d=D), op=OP.subtract)
            sqd = p_dve.tile([128, 512], BF16, tag="sqd")
            nc.scalar.activation(sqd[:], diff[:], AF.Square)
            d2 = p_dve.tile([128, 16], F32, tag="d2")
            nc.vector.reduce_sum(
                out=d2[:], in_=sqd[:].rearrange("p (j d) -> p j d", d=D),
                axis=AX.X,
            )
            dist = p_dve.tile([128, 16], F32, tag="dist")
            nc.scalar.activation(dist[:], d2[:], AF.Sqrt, bias=eps_b[:])
            hcol = g * 32 + half * 16
            nc.scalar.activation(h_all[:, hcol:hcol + 16], dist[:],
                                 AF.Relu, bias=ndv_b[:])

        if EMIT_PHASE != "ab" and prev_oh is not None:
            emit_phase_c(prev_oh, prev_g)
        prev_oh, prev_g = ohb, g
    if EMIT_PHASE == "ab":
        res_sb = p_small.tile([1, 8], F32, tag="res_sb")
        nc.vector.memset(res_sb[:], 0.0)
        nc.vector.tensor_copy(res_sb[:, 0:1], h_all[0:1, 0:1])
        nc.sync.dma_start(res_d[:], res_sb[:])
        return
    emit_phase_c(prev_oh, prev_g)

    if EMIT_PHASE == "abc":
        res_sb = p_small.tile([1, 8], F32, tag="res_sb")
        nc.vector.memset(res_sb[:], 0.0)
        nc.vector.tensor_copy(res_sb[:, 0:1], psum_hc[0:1, :])
        nc.sync.dma_start(res_d[:], res_sb[:])
        return

    # ================= finals =================
    seg_mean = p_small.tile([32, 1], F32, tag="seg_mean")
    nc.vector.tensor_scalar(out=seg_mean[:], in0=psum_hc[:], scalar1=recip[:],
                            scalar2=None, op0=OP.mult)

    cat4 = p_small.tile([32, 4], F32, tag="cat4")
    nc.vector.tensor_copy(cat4[:, 0:1], seg_mean[:])
    nc.vector.tensor_copy(cat4[:, 1:2], present[:])
    nc.vector.tensor_copy(cat4[:, 2:3], hp_rs[:])
    nc.vector.tensor_copy(cat4[:, 3:4], pm_rs[:])
    ps_fin = ps_misc.tile([1, 4], F32, tag="misc")
    nc.tensor.matmul(ps_fin[:], ones32c[:], cat4[:], start=True, stop=True)
    sc = p_small.tile([1, 4], F32, tag="sc")
    nc.vector.tensor_copy(sc[:], ps_fin[:])

    res_sb = p_small.tile([1, 8], F32, tag="res_sb")
    nc.vector.memset(res_sb[:], 0.0)
    t1 = p_small.tile([1, 1], F32, tag="t1")
    nc.vector.tensor_scalar(out=t1[:], in0=sc[:, 1:2], scalar1=1e-6,
                            scalar2=None, op0=OP.add)
    r1 = p_small.tile([1, 1], F32, tag="r1")
    nc.vector.reciprocal(r1[:], t1[:])
    nc.vector.tensor_tensor(out=res_sb[:, 0:1], in0=sc[:, 0:1], in1=r1[:],
                            op=OP.mult)
    t2 = p_small.tile([1, 1], F32, tag="t2")
    nc.vector.tensor_scalar(out=t2[:], in0=sc[:, 3:4], scalar1=1e-6,
                            scalar2=None, op0=OP.add)
    r2 = p_small.tile([1, 1], F32, tag="r2")
    nc.vector.reciprocal(r2[:], t2[:])
    pb0 = p_small.tile([1, 1], F32, tag="pb0")
    nc.vector.tensor_tensor(out=pb0[:], in0=sc[:, 2:3], in1=r2[:], op=OP.mult)
    gate = p_small.tile([1, 1], F32, tag="gate")
    nc.vector.tensor_scalar(out=gate[:], in0=sc[:, 1:2], scalar1=1.0,
                            scalar2=None, op0=OP.is_gt)
    nc.vector.tensor_tensor(out=res_sb[:, 1:2], in0=pb0[:], in1=gate[:],
                            op=OP.mult)

    nc.sync.dma_start(res_d[:], res_sb[:])


def build_program(groups):
    n = groups * 4096
    nc = bacc.Bacc("TRN2", target_bir_lowering=False, debug=False)
    emb_d = nc.dram_tensor("emb", [n, D], F32, kind="ExternalInput")
    lab_d = nc.dram_tensor("lab", [n], I32, kind="ExternalInput")
    res_d = nc.dram_tensor("res", [1, 8], F32, kind="ExternalOutput")
    with tile.TileContext(nc) as tc:
        with ExitStack() as ctx:
            tc.ctx = ctx
            emit(tc, emb_d, lab_d, res_d, groups)
    nc.compile()
    return nc


_NC_CACHE = {}


def _get_nc(groups):
    if groups not in _NC_CACHE:
        _NC_CACHE[groups] = build_program(groups)
    return _NC_CACHE[groups]


def kernel(embeddings, labels):
    embeddings = np.asarray(embeddings, dtype=np.float32)
    labels = np.asarray(labels, dtype=np.int32)
    bsz = embeddings.shape[0]
    groups = embeddings.shape[1] // 4096
    nc = _get_nc(groups)

    from concourse.bass_utils import run_bass_kernel_spmd

    in_maps = [
        {"emb": np.ascontiguousarray(embeddings[b]),
         "lab": np.ascontiguousarray(labels[b])}
        for b in range(bsz)
    ]
    out = run_bass_kernel_spmd(nc, in_maps, list(range(bsz)))
    res = np.stack([out.results[b]["res"][0] for b in range(bsz)])
    pull = res[:, 0].sum() / bsz
    push = res[:, 1].sum() / bsz
    return np.stack([pull + push, pull, push]).astype(np.float32)


# revision 18
# speedup vs baseline: 1.1480x; 1.1152x over previous
"""DiscriminativeLoss Trainium2 kernel (Bass/Tile), data-parallel over batch.

Per core: one batch element [N=131072, D=32] f32 + labels [N] i32.
Returns per-core partial losses (pull_b, push_b); host averages over the
8 cores and assembles [total, pull, push].

v2: all matmul sweeps are one LDW+MM pair per 128-point chunk:
  A: lhsT=oh_bf[128,32], rhs=[hi|lo|ones] F=65 -> segment sums (hi/lo
     bf16 split, ~2^-17 accurate) + exact counts.
  B: lhsT=ohT_full[32,128] (FWL bf16), rhs=means F=32 -> per-point mean.
  C: lhsT=oh_bf, rhs=hinge F=1 -> per-label hinge sums.
"""

import os
import sys

sys.path.insert(0, "/opt/trn_rl_repo")

import numpy as np
from contextlib import ExitStack

import concourse.bass as bass
import concourse.bacc as bacc
import concourse.mybir as mybir
import concourse.tile as tile

F32 = mybir.dt.float32
BF16 = mybir.dt.bfloat16
I32 = mybir.dt.int32
AX = mybir.AxisListType
OP = mybir.AluOpType
AF = mybir.ActivationFunctionType

B, N_FULL, D = 8, 131072, 32
EMIT_PHASE = "full"   # "a" | "ab" | "abc" | "full"  (bisect aid)
NL = 32          # instance labels 1..32 (label 0 ignored)
DELTA_V = 0.1
DELTA_D = 0.5
HL = 2 * D + 2   # 66: hi(32) | lo(32) | ones(1) | pad, 4B-aligned stride


def emit(tc, emb_d, lab_d, res_d, groups):
    nc = tc.nc
    ctx = tc.ctx
    npc = groups * 32           # points per partition

    emb_v = emb_d[:].rearrange("(p c) d -> p (c d)", p=128)
    lab_v = lab_d[:].rearrange("(p c) -> p c", p=128)

    # ---------------- pools ----------------
    p_in = ctx.enter_context(tc.tile_pool(name="p_in", bufs=3))
    p_ohb = ctx.enter_context(tc.tile_pool(name="p_ohb", bufs=3))
    p_oht = ctx.enter_context(tc.tile_pool(name="p_oht", bufs=3))
    p_ohf = ctx.enter_context(tc.tile_pool(name="p_ohf", bufs=3))
    p_pers = ctx.enter_context(tc.tile_pool(name="p_pers", bufs=1))
    p_small = ctx.enter_context(tc.tile_pool(name="p_small", bufs=1))
    p_dve = ctx.enter_context(tc.tile_pool(name="p_dve", bufs=3))
    ps_a = ctx.enter_context(tc.tile_pool(name="ps_a", bufs=1, space="PSUM"))
    ps_hc = ctx.enter_context(tc.tile_pool(name="ps_hc", bufs=1, space="PSUM"))
    ps_mp = ctx.enter_context(tc.tile_pool(name="ps_mp", bufs=3, space="PSUM"))
    ps_misc = ctx.enter_context(tc.tile_pool(name="ps_misc", bufs=3, space="PSUM"))

    # ---------------- constants / persistent ----------------
    lab_i = p_in.tile([128, npc], I32, tag="lab_i")
    nc.sync.dma_start(lab_i[:], lab_v)
    lab_b = p_pers.tile([128, npc], BF16, tag="lab_b")
    nc.vector.tensor_copy(lab_b[:], lab_i[:])

    iota_i = p_small.tile([128, NL], I32, tag="iota_i")
    nc.gpsimd.iota(iota_i[:], pattern=[[1, NL]], base=1, channel_multiplier=0)
    iota_b = p_small.tile([128, NL], BF16, tag="iota_b")
    nc.vector.tensor_copy(iota_b[:], iota_i[:])

    # 32x32 identity (f32)
    ones32 = p_small.tile([32, 32], F32, tag="ones32")
    nc.vector.memset(ones32[:], 1.0)
    id32 = p_small.tile([32, 32], F32, tag="id32")
    nc.gpsimd.affine_select(
        id32[:], ones32[:], pattern=[[1, 32]], base=0,
        channel_multiplier=-1, compare_op=OP.is_equal, fill=0.0,
    )
    ones_k1 = p_small.tile([1, 32], F32, tag="ones_k1")
    nc.vector.memset(ones_k1[:], 1.0)
    ones32c = p_small.tile([32, 1], F32, tag="ones32c")
    nc.vector.memset(ones32c[:], 1.0)
    eps_b = p_small.tile([128, 1], F32, tag="eps_b")
    nc.vector.memset(eps_b[:], 1e-24)
    ndv_b = p_small.tile([128, 1], F32, tag="ndv_b")
    nc.vector.memset(ndv_b[:], -DELTA_V)

    # persistent big buffers
    emb_hl = p_pers.tile([128, npc * HL], BF16, tag="emb_hl")
    h_all = p_pers.tile([128, npc], BF16, tag="h_all")

    psum_a = ps_a.tile([32, HL], F32, tag="psum_a")
    psum_hc = ps_hc.tile([32, 1], F32, tag="psum_hc")

    # strided views of emb_hl: [p, point, slot]
    hl3 = emb_hl[:].rearrange("p (c k) -> p c k", k=HL)

    # ones (col 64) + pad (col 65) of each 66-block, set once
    nc.vector.memset(hl3[:, :, 2 * D:2 * D + 2], 1.0)

    # ================= PHASE A: segment sums + counts =================
    for g in range(groups):
        ta = p_in.tile([128, 1024], F32, tag="ta")
        nc.sync.dma_start(ta[:], emb_v[:, g * 1024:(g + 1) * 1024])
        ta3 = ta[:].rearrange("p (c d) -> p c d", d=D)

        oh = p_ohb.tile([128, 1024], BF16, tag="ohb")
        in0 = lab_b[:, g * 32:(g + 1) * 32].unsqueeze(2).broadcast_to([128, 32, NL])
        in1 = iota_b[:].unsqueeze(1).broadcast_to([128, 32, NL])
        oh3 = oh[:].rearrange("p (j l) -> p j l", l=NL)
        nc.vector.tensor_tensor(out=oh3, in0=in0, in1=in1, op=OP.is_equal)

        # hi = bf16(e) (ACT), lo = bf16(e - hi) (DVE)
        hi3 = hl3[:, g * 32:(g + 1) * 32, 0:D]
        lo3 = hl3[:, g * 32:(g + 1) * 32, D:2 * D]
        nc.scalar.copy(hi3, ta3)
        nc.vector.tensor_tensor(out=lo3, in0=ta3, in1=hi3, op=OP.subtract)

        for j in range(32):
            cj = g * 32 + j
            nc.tensor.matmul(
                psum_a[:], oh[:, j * NL:(j + 1) * NL],
                emb_hl[:, cj * HL:(cj + 1) * HL],
                start=(cj == 0), stop=(cj == groups * 32 - 1),
            )

    # ================= means & push tail (tiny, f32) =================
    cnt = psum_a[:, 2 * D:2 * D + 1]
    cnt_cl = p_small.tile([32, 1], F32, tag="cnt_cl")
    nc.vector.tensor_scalar(out=cnt_cl[:], in0=cnt, scalar1=1.0,
                            scalar2=None, op0=OP.max)
    recip = p_small.tile([32, 1], F32, tag="recip")
    nc.vector.reciprocal(recip[:], cnt_cl[:])
    suml_sb = p_small.tile([32, 32], F32, tag="suml_sb")
    nc.vector.tensor_copy(suml_sb[:], psum_a[:, D:2 * D])
    sums_f = p_small.tile([32, 32], F32, tag="sums_f")
    nc.vector.tensor_tensor(out=sums_f[:], in0=psum_a[:, 0:D],
                            in1=suml_sb[:], op=OP.add)
    means_f = p_small.tile([32, 32], F32, tag="means_f")
    nc.vector.tensor_scalar(out=means_f[:], in0=sums_f[:],
                            scalar1=recip[:], scalar2=None, op0=OP.mult)
    means_b = p_small.tile([32, 32], BF16, tag="means_b")
    nc.vector.tensor_copy(means_b[:], means_f[:])

    if EMIT_PHASE == "a":
        res_sb = p_small.tile([1, 8], F32, tag="res_sb")
        nc.vector.memset(res_sb[:], 0.0)
        nc.vector.tensor_copy(res_sb[:, 0:1], cnt_cl[0:1, :])
        nc.sync.dma_start(res_d[:], res_sb[:])
        return

    # --- push loss on the 32x32 mean matrix ---
    mnsq = p_small.tile([32, 32], F32, tag="mnsq")
    nc.vector.tensor_tensor(out=mnsq[:], in0=means_f[:], in1=means_f[:], op=OP.mult)
    nrm2 = p_small.tile([32, 1], F32, tag="nrm2")
    nc.vector.reduce_sum(out=nrm2[:], in_=mnsq[:], axis=AX.X)
    nrm = p_small.tile([32, 1], F32, tag="nrm")
    nc.scalar.activation(nrm[:], nrm2[:], AF.Sqrt)
    nrm_cl = p_small.tile([32, 1], F32, tag="nrm_cl")
    nc.vector.tensor_scalar(out=nrm_cl[:], in0=nrm[:], scalar1=1e-12,
                            scalar2=None, op0=OP.max)
    rnrm = p_small.tile([32, 1], F32, tag="rnrm")
    nc.vector.reciprocal(rnrm[:], nrm_cl[:])
    mn = p_small.tile([32, 32], F32, tag="mn")
    nc.vector.tensor_scalar(out=mn[:], in0=means_f[:], scalar1=rnrm[:],
                            scalar2=None, op0=OP.mult)

    ps_mnt = ps_misc.tile([32, 32], F32, tag="misc")
    nc.tensor.transpose(ps_mnt[:], mn[:], id32[:])
    mnt = p_small.tile([32, 32], F32, tag="mnt")
    nc.vector.tensor_copy(mnt[:], ps_mnt[:])

    ps_g = ps_misc.tile([32, 32], F32, tag="misc")
    nc.tensor.matmul(ps_g[:], mnt[:], mnt[:], start=True, stop=True)

    mnsq2 = p_small.tile([32, 32], F32, tag="mnsq2")
    nc.vector.tensor_tensor(out=mnsq2[:], in0=mn[:], in1=mn[:], op=OP.mult)
    nsq = p_small.tile([32, 1], F32, tag="nsq")
    nc.vector.reduce_sum(out=nsq[:], in_=mnsq2[:], axis=AX.X)

    present = p_small.tile([32, 1], F32, tag="present")
    nc.vector.tensor_scalar(out=present[:], in0=cnt, scalar1=0.0,
                            scalar2=None, op0=OP.is_gt)

    sq_a = p_small.tile([32, 32], F32, tag="sq_a")
    nc.vector.tensor_scalar(out=sq_a[:], in0=ps_g[:], scalar1=-2.0,
                            scalar2=nsq[:], op0=OP.mult, op1=OP.add)

    ps_row0 = ps_misc.tile([1, 32], F32, tag="misc")
    nc.tensor.matmul(ps_row0[:], nsq[:], id32[:], start=True, stop=True)
    nsqt_sb = p_small.tile([1, 32], F32, tag="nsqt_sb")
    nc.vector.tensor_copy(nsqt_sb[:], ps_row0[:])
    ps_row1 = ps_misc.tile([1, 32], F32, tag="misc")
    nc.tensor.matmul(ps_row1[:], present[:], id32[:], start=True, stop=True)
    prest_sb = p_small.tile([1, 32], F32, tag="prest_sb")
    nc.vector.tensor_copy(prest_sb[:], ps_row1[:])

    ps_bc = ps_misc.tile([32, 64], F32, tag="misc")
    nc.tensor.matmul(ps_bc[:, 0:32], ones_k1[:], nsqt_sb[:],
                     start=True, stop=True)
    nc.tensor.matmul(ps_bc[:, 32:64], ones_k1[:], prest_sb[:],
                     start=True, stop=True)
    nsq_j = p_small.tile([32, 32], F32, tag="nsq_j")
    nc.vector.tensor_copy(nsq_j[:], ps_bc[:, 0:32])
    pres_j = p_small.tile([32, 32], F32, tag="pres_j")
    nc.vector.tensor_copy(pres_j[:], ps_bc[:, 32:64])

    sq0 = p_small.tile([32, 32], F32, tag="sq0")
    nc.vector.tensor_tensor(out=sq0[:], in0=sq_a[:], in1=nsq_j[:], op=OP.add)
    sq = p_small.tile([32, 32], F32, tag="sq")
    nc.vector.tensor_scalar(out=sq[:], in0=sq0[:], scalar1=0.0,
                            scalar2=None, op0=OP.max)
    dmat = p_small.tile([32, 32], F32, tag="dmat")
    nc.scalar.activation(dmat[:], sq[:], AF.Sqrt, bias=eps_b[0:32, :])
    hp0 = p_small.tile([32, 32], F32, tag="hp0")
    nc.scalar.activation(hp0[:], dmat[:], AF.Relu, bias=ones32c[:], scale=-1.0)
    hp1 = p_small.tile([32, 32], F32, tag="hp1")
    nc.vector.tensor_scalar(out=hp1[:], in0=hp0[:], scalar1=present[:],
                            scalar2=None, op0=OP.mult)
    hp2 = p_small.tile([32, 32], F32, tag="hp2")
    nc.vector.tensor_tensor(out=hp2[:], in0=hp1[:], in1=pres_j[:], op=OP.mult)
    hp3 = p_small.tile([32, 32], F32, tag="hp3")
    nc.gpsimd.affine_select(hp3[:], hp2[:], pattern=[[1, 32]], base=0,
                            channel_multiplier=-1, compare_op=OP.is_gt, fill=0.0)
    pm1 = p_small.tile([32, 32], F32, tag="pm1")
    nc.vector.tensor_scalar(out=pm1[:], in0=pres_j[:], scalar1=present[:],
                            scalar2=None, op0=OP.mult)
    pm = p_small.tile([32, 32], F32, tag="pm")
    nc.gpsimd.affine_select(pm[:], pm1[:], pattern=[[1, 32]], base=0,
                            channel_multiplier=-1, compare_op=OP.is_gt, fill=0.0)
    hp_rs = p_small.tile([32, 1], F32, tag="hp_rs")
    nc.vector.reduce_sum(out=hp_rs[:], in_=hp3[:], axis=AX.X)
    pm_rs = p_small.tile([32, 1], F32, tag="pm_rs")
    nc.vector.reduce_sum(out=pm_rs[:], in_=pm[:], axis=AX.X)

    # ================= PHASE B (+ pipelined C): pull loss =================
    prev_oh = None
    prev_g = None
    first_c = [True]

    def emit_phase_c(ohb_t, g):
        for j in range(32):
            cj = g * 32 + j
            nc.tensor.matmul(
                psum_hc[:], ohb_t[:, j * NL:(j + 1) * NL],
                h_all[:, cj:cj + 1],
                start=first_c[0], stop=(cj == groups * 32 - 1),
            )
            first_c[0] = False

    for g in range(groups):
        ohb = p_ohb.tile([128, 1024], BF16, tag="ohb")
        in0 = lab_b[:, g * 32:(g + 1) * 32].unsqueeze(2).broadcast_to([128, 32, NL])
        in1 = iota_b[:].unsqueeze(1).broadcast_to([128, 32, NL])
        ohb3 = ohb[:].rearrange("p (j l) -> p j l", l=NL)
        nc.vector.tensor_tensor(out=ohb3, in0=in0, in1=in1, op=OP.is_equal)

        oht = p_oht.tile([128, 1024], BF16, tag="oht")
        nc.vector.transpose(oht[:], ohb[:])

        # rearrange 32x32 blocks: ohT_full[0:32, 128j+32b+s] <- ohT block
        ohf = p_ohf.tile([32, 4096], BF16, tag="ohf")
        ohf3 = ohf[:].rearrange("p (j b s) -> p j b s", b=4, s=32)
        oht3 = oht[:].rearrange("p (j s) -> p j s", s=32)
        for b4 in range(4):
            eng = nc.sync if b4 % 2 == 0 else nc.scalar
            eng.dma_start(ohf3[:, :, b4, :],
                          oht3[32 * b4:32 * b4 + 32, :, :])

        for half in range(2):
            mp = ps_mp.tile([128, 512], F32, tag="mp")
            for jj in range(16):
                j = half * 16 + jj
                nc.tensor.matmul(
                    mp[:, jj * 32:(jj + 1) * 32],
                    ohf[:, j * 128:(j + 1) * 128],
                    means_b[:],
                    start=True, stop=True,
                )
            base3 = hl3[:, g * 32 + half * 16:g * 32 + (half + 1) * 16, 0:D]
            diff = p_dve.tile([128, 512], BF16, tag="diff")
            diff3 = diff[:].rearrange("p (j d) -> p j d", d=D)
            nc.vector.tensor_tensor(out=diff3, in0=base3, in1=mp[:].rearrange(
                "p (j d) -> p j d", d=D), op=OP.subtract)
            sqd = p_dve.tile([128, 512], BF16, tag="sqd")
            nc.scalar.activation(sqd[:], diff[:], AF.Square)
            d2 = p_dve.tile([128, 16], F32, tag="d2")
            nc.vector.reduce_sum(
                out=d2[:], in_=sqd[:].rearrange("p (j d) -> p j d", d=D),
                axis=AX.X,
            )
            dist = p_dve.tile([128, 16], F32, tag="dist")
            nc.scalar.activation(dist[:], d2[:], AF.Sqrt, bias=eps_b[:])
            hcol = g * 32 + half * 16
            nc.scalar.activation(h_all[:, hcol:hcol + 16], dist[:],
                                 AF.Relu, bias=ndv_b[:])

        if EMIT_PHASE != "ab" and prev_oh is not None:
            emit_phase_c(prev_oh, prev_g)
        prev_oh, prev_g = ohb, g
    if EMIT_PHASE == "ab":
        res_sb = p_small.tile([1, 8], F32, tag="res_sb")
        nc.vector.memset(res_sb[:], 0.0)
        nc.vector.tensor_copy(res_sb[:, 0:1], h_all[0:1, 0:1])
        nc.sync.dma_start(res_d[:], res_sb[:])
        return
    emit_phase_c(prev_oh, prev_g)

    if EMIT_PHASE == "abc":
        res_sb = p_small.tile([1, 8], F32, tag="res_sb")
        nc.vector.memset(res_sb[:], 0.0)
        nc.vector.tensor_copy(res_sb[:, 0:1], psum_hc[0:1, :])
        nc.sync.dma_start(res_d[:], res_sb[:])
        return

    # ================= finals =================
    seg_mean = p_small.tile([32, 1], F32, tag="seg_mean")
    nc.vector.tensor_scalar(out=seg_mean[:], in0=psum_hc[:], scalar1=recip[:],
                            scalar2=None, op0=OP.mult)

    cat4 = p_small.tile([32, 4], F32, tag="cat4")
    nc.vector.tensor_copy(cat4[:, 0:1], seg_mean[:])
    nc.vector.tensor_copy(cat4[:, 1:2], present[:])
    nc.vector.tensor_copy(cat4[:, 2:3], hp_rs[:])
    nc.vector.tensor_copy(cat4[:, 3:4], pm_rs[:])
    ps_fin = ps_misc.tile([1, 4], F32, tag="misc")
    nc.tensor.matmul(ps_fin[:], ones32c[:], cat4[:], start=True, stop=True)
    sc = p_small.tile([1, 4], F32, tag="sc")
    nc.vector.tensor_copy(sc[:], ps_fin[:])

    res_sb = p_small.tile([1, 8], F32, tag="res_sb")
    nc.vector.memset(res_sb[:], 0.0)
    t1 = p_small.tile([1, 1], F32, tag="t1")
    nc.vector.tensor_scalar(out=t1[:], in0=sc[:, 1:2], scalar1=1e-6,
                            scalar2=None, op0=OP.add)
    r1 = p_small.tile([1, 1], F32, tag="r1")
    nc.vector.reciprocal(r1[:], t1[:])
    nc.vector.tensor_tensor(out=res_sb[:, 0:1], in0=sc[:, 0:1], in1=r1[:],
                            op=OP.mult)
    t2 = p_small.tile([1, 1], F32, tag="t2")
    nc.vector.tensor_scalar(out=t2[:], in0=sc[:, 3:4], scalar1=1e-6,
                            scalar2=None, op0=OP.add)
    r2 = p_small.tile([1, 1], F32, tag="r2")
    nc.vector.reciprocal(r2[:], t2[:])
    pb0 = p_small.tile([1, 1], F32, tag="pb0")
    nc.vector.tensor_tensor(out=pb0[:], in0=sc[:, 2:3], in1=r2[:], op=OP.mult)
    gate = p_small.tile([1, 1], F32, tag="gate")
    nc.vector.tensor_scalar(out=gate[:], in0=sc[:, 1:2], scalar1=1.0,
                            scalar2=None, op0=OP.is_gt)
    nc.vector.tensor_tensor(out=res_sb[:, 1:2], in0=pb0[:], in1=gate[:],
                            op=OP.mult)

    nc.sync.dma_start(res_d[:], res_sb[:])


def build_program(groups):
    n = groups * 4096
    nc = bacc.Bacc("TRN2", target_bir_lowering=False, debug=False)
    emb_d = nc.dram_tensor("emb", [n, D], F32, kind="ExternalInput")
    lab_d = nc.dram_tensor("lab", [n], I32, kind="ExternalInput")
    res_d = nc.dram_tensor("res", [1, 8], F32, kind="ExternalOutput")
    with tile.TileContext(nc) as tc:
        with ExitStack() as ctx:
            tc.ctx = ctx
            emit(tc, emb_d, lab_d, res_d, groups)
    nc.compile()
    return nc


_NC_CACHE = {}


def _get_nc(groups):
    if groups not in _NC_CACHE:
        _NC_CACHE[groups] = build_program(groups)
    return _NC_CACHE[groups]


def kernel(embeddings, labels):
    embeddings = np.asarray(embeddings, dtype=np.float32)
    labels = np.asarray(labels, dtype=np.int32)
    bsz = embeddings.shape[0]
    groups = embeddings.shape[1] // 4096
    nc = _get_nc(groups)

    from concourse.bass_utils import run_bass_kernel_spmd

    in_maps = [
        {"emb": np.ascontiguousarray(embeddings[b]),
         "lab": np.ascontiguousarray(labels[b])}
        for b in range(bsz)
    ]
    out = run_bass_kernel_spmd(nc, in_maps, list(range(bsz)))
    res = np.stack([out.results[b]["res"][0] for b in range(bsz)])
    pull = res[:, 0].sum() / bsz
    push = res[:, 1].sum() / bsz
    return np.stack([pull + push, pull, push]).astype(np.float32)
